# revision 16
# baseline (speedup 1.0000x reference)
"""Trainium2 Bass kernel for nn_Loc2Cluster (GNN message passing, segment-max).

Computation: agg[c] = elementwise-max over locs with edge to cluster c of
x_locs[loc]; empty clusters -> 0; output = concat([x_clusters, agg], -1).

Strategy (cluster-sharded, zero collectives, bf16 streams):
  - Core k owns clusters [4096k, 4096(k+1)) after a global count-desc sort
    dealt round-robin across cores (balances per-core round sizes to +-1).
  - Host routes each edge's loc row (pre-converted to bf16; rel tol is
    2e-2, bf16 rounding is ~2^-9) to the core owning its dst cluster.
  - Rows are laid out in "rounds": round r holds the r-th edge row of every
    cluster with count > r, in local-rank order (chunk-major), so each
    round's segment-max is elementwise tensor_max over a prefix of slots.
  - Big rounds (m_r > 128) stream exactly m_r rows: full chunks as one
    chunk-major DMA + the partial last chunk as a small second DMA, each
    followed by its own tensor_max. No pad bytes.
  - Tail rounds (m_r <= 128) all touch only chunk 0; consecutive ones are
    merged into grouped DMA blocks (padded to the group max with -1e30,
    which is max-neutral) and are scheduled right after round 0 so their
    serial max chains hide under the big-round stream DMAs.
  - The last big round is split so the final flush waits only on one small
    [128, 256] tensor_max.
  - Round 0 is DMA'd straight into the accumulator (zero rows for empty
    clusters match the reference's 0-fill).
  - Output chunks are written back (scalar-engine queue) as soon as no
    later item touches them, overlapping writeback with the stream.
  - x_clusters never touches the device: the concat left half is assembled
    on the host at full fp32 precision during unsharding.
"""

import sys

import numpy as np

if "/opt/trn_rl_repo" not in sys.path:
    sys.path.insert(0, "/opt/trn_rl_repo")

import ml_dtypes

BF16 = ml_dtypes.bfloat16

N_LOCS = 262144
N_CLUSTERS = 32768
D = 256
N_CORES = 8
CPC = N_CLUSTERS // N_CORES  # 4096 clusters per core
P = 128
CHUNKS = CPC // P  # 32 chunks of 128 clusters
NEG = np.float32(-1e30)

LAST_RESULTS = None  # BassKernelResults of the most recent run (for profiling)
LAST_NC = None  # compiled Bass module of the most recent run (for TimelineSim)


def _make_schedule(m):
    """Turn per-core round sizes m[r] into an execution schedule.

    Returns (items, offs_of_round, TOT). items are in execution order:
      ("R0",)                      round 0, CPC rows into the accumulator
      ("A", off, mr)               big round, exact mr rows (q full chunks
                                   chunk-major + rem-row partial chunk)
      ("AB", off, q, rem)          partial-chunk piece of the split last round
      ("AA", off, q)               full-chunk piece of the split last round
      ("G", off, Mstar, members)   merged tail rounds, members=[(r, m_r)]
    Stream layout (HBM row offsets) is in round order; execution order is
    [R0, tail groups..., big rounds desc, last round's B then A piece].
    """
    R = len(m)
    offs_of_round = np.zeros(R, dtype=np.int64)
    a_items = []
    g_items = []
    pos = 0
    r = 1
    offs_of_round[0] = 0
    pos = CPC
    while r < R:
        if m[r] > P:
            offs_of_round[r] = pos
            a_items.append(("A", int(pos), int(m[r])))
            pos += int(m[r])
            r += 1
        else:
            # greedy tail group of consecutive small rounds; new group when
            # padding to the group max would get wasteful
            Mstar = int(m[r])
            members = []
            gpos = pos
            while r < R and m[r] <= P and m[r] * 4 >= Mstar and len(members) < 8:
                members.append((r, int(m[r])))
                offs_of_round[r] = gpos
                gpos += Mstar
                r += 1
            g_items.append(("G", int(pos), Mstar, members))
            pos = gpos
    items = [("R0",)] + g_items + a_items
    # split the last big round so the final flush gates on one small max
    if a_items:
        kind, off, mr = a_items[-1]
        q, rem = divmod(mr, P)
        if q > 0:
            items = items[:-1]
            if rem > 0:
                items.append(("AB", off + q * P, q, rem))
            items.append(("AA", off, q))
    return items, offs_of_round, int(pos)


def _host_prep(x_locs, x_clusters, edge_src, edge_dst):
    """Build per-core bf16 round-major row streams."""
    x_locs = np.asarray(x_locs, dtype=np.float32)
    x_bf = x_locs.astype(BF16)
    src = np.asarray(edge_src).astype(np.int64)
    dst = np.asarray(edge_dst).astype(np.int64)
    n_edges = dst.shape[0]

    counts = np.bincount(dst, minlength=N_CLUSTERS)  # [32768]

    gorder = np.argsort(-counts, kind="stable")
    grank = np.empty_like(gorder)
    grank[gorder] = np.arange(N_CLUSTERS)
    order = np.ascontiguousarray(gorder.reshape(CPC, N_CORES).T)  # [8, CPC]

    # occurrence index of each edge within its dst cluster
    by_dst = np.argsort(dst, kind="stable")
    group_start = np.zeros(N_CLUSTERS, dtype=np.int64)
    np.cumsum(counts[:-1], out=group_start[1:])
    occ = np.empty(n_edges, dtype=np.int64)
    occ[by_dst] = np.arange(n_edges, dtype=np.int64) - group_start[dst[by_dst]]

    g_of = grank[dst]
    core_of = g_of % N_CORES
    rank_of = g_of // N_CORES  # local rank s, count-sorted desc

    R = max(int(counts.max()), 1)
    counts_sorted = counts[gorder]
    m_r_g = (counts_sorted[None, :] > np.arange(R)[:, None]).sum(axis=1)
    m = (m_r_g + N_CORES - 1) // N_CORES  # shared per-core round size
    m[0] = CPC  # round 0 covers every slot (zeros for empty clusters)

    items, offs_of_round, TOT = _make_schedule(m)

    # chunk-major blocks everywhere: slot within a round block == local rank
    slot = offs_of_round[occ] + rank_of

    slot_src = np.full((N_CORES, TOT), -1, dtype=np.int64)
    slot_src[core_of, slot] = src

    in_maps = []
    for k in range(N_CORES):
        ss = slot_src[k]
        stream = x_bf[np.maximum(ss, 0)]  # [TOT, 256] bf16
        pad = ss < 0
        p0 = np.flatnonzero(pad[:CPC])
        if p0.size:
            stream[p0] = 0.0  # empty clusters -> 0
        pr = np.flatnonzero(pad[CPC:]) + CPC
        if pr.size:
            stream[pr] = NEG  # pads (group + straggler) are max-neutral
        in_maps.append({"rows": np.ascontiguousarray(stream)})

    return in_maps, order, items, TOT


def _build_program(items, TOT, bufs=9):
    from concourse import bacc, mybir
    from concourse._compat import axon_active
    from concourse.tile import TileContext

    nc = bacc.Bacc(
        "TRN2",
        target_bir_lowering=False,
        debug=not axon_active(),
        num_devices=N_CORES,
    )
    rows_h = nc.dram_tensor("rows", [TOT, D], mybir.dt.bfloat16, kind="ExternalInput")
    out_h = nc.dram_tensor(
        "out", [P, CHUNKS, D], mybir.dt.bfloat16, kind="ExternalOutput"
    )

    # per-chunk last-toucher item index, for early writeback
    def touched(it):
        if it[0] == "R0":
            return range(CHUNKS)
        if it[0] == "A":
            q, rem = divmod(it[2], P)
            return range(q + (1 if rem else 0))
        if it[0] == "AB":
            return range(it[2], it[2] + 1)
        if it[0] == "AA":
            return range(it[2])
        return range(1)  # G: chunk 0 only

    last_touch = [0] * CHUNKS
    for i, it in enumerate(items):
        for c in touched(it):
            last_touch[c] = i

    with TileContext(nc) as tc:
        with (
            tc.tile_pool(name="accp", bufs=1) as accp,
            tc.tile_pool(name="stagep", bufs=bufs) as stagep,
            tc.tile_pool(name="smallp", bufs=4) as smallp,
        ):
            acc = accp.tile([P, CHUNKS * D], mybir.dt.bfloat16)
            acc3 = acc[:].rearrange("p (x f) -> p x f", f=D)
            pending_hi = CHUNKS

            def flush(i):
                nonlocal pending_hi
                lo = pending_hi
                while lo > 0 and last_touch[lo - 1] <= i:
                    lo -= 1
                # merge tiny writebacks mid-stream, but flush eagerly near
                # the end so the final flush gates on as little as possible
                eager = i >= len(items) - 4
                final = i == len(items) - 1
                if lo < pending_hi and (eager or pending_hi - lo >= 3):
                    # a DMA's sem wait holds its SEQ: mid-stream writebacks
                    # go on the otherwise-idle scalar queue so they never
                    # stall stream-DMA issue; the final flush goes on SP
                    # (free by then, and its issue path is ~140ns shorter)
                    eng = nc.sync if final else nc.scalar
                    eng.dma_start(
                        out=out_h.ap()[:, lo:pending_hi, :],
                        in_=acc3[:, lo:pending_hi, :],
                    )
                    pending_hi = lo

            def dma_a_piece(dst, off, q):
                blk = rows_h.ap()[off : off + q * P].rearrange(
                    "(x p) f -> p x f", p=P
                )
                dst3 = dst[:, : q * D].rearrange("p (x f) -> p x f", f=D)
                nc.sync.dma_start(out=dst3, in_=blk)

            def dma_b_piece(dst, off, q, rem):
                nc.sync.dma_start(
                    out=dst[0:rem, q * D : (q + 1) * D],
                    in_=rows_h.ap()[off : off + rem],
                )

            for i, it in enumerate(items):
                if it[0] == "R0":
                    dma_a_piece(acc, 0, CHUNKS)
                elif it[0] == "A":
                    _, off, mr = it
                    q, rem = divmod(mr, P)
                    st = stagep.tile([P, CHUNKS * D], mybir.dt.bfloat16, tag="st")
                    if q > 0:
                        dma_a_piece(st, off, q)
                    if rem > 0:
                        dma_b_piece(st, off + q * P, q, rem)
                    if q > 0:
                        w = q * D
                        nc.vector.tensor_max(
                            out=acc[:, :w], in0=acc[:, :w], in1=st[:, :w]
                        )
                    if rem > 0:
                        sl = slice(q * D, (q + 1) * D)
                        nc.vector.tensor_max(
                            out=acc[0:rem, sl], in0=acc[0:rem, sl], in1=st[0:rem, sl]
                        )
                elif it[0] == "AB":
                    _, off, q, rem = it
                    st = smallp.tile([P, 8 * D], mybir.dt.bfloat16, tag="sts")
                    dma_b_piece(st, off, q, rem)
                    sl = slice(q * D, (q + 1) * D)
                    nc.vector.tensor_max(
                        out=acc[0:rem, sl], in0=acc[0:rem, sl], in1=st[0:rem, sl]
                    )
                elif it[0] == "AA":
                    _, off, q = it
                    st = smallp.tile([P, 8 * D], mybir.dt.bfloat16, tag="sts")
                    dma_a_piece(st, off, q)
                    w = q * D
                    nc.vector.tensor_max(
                        out=acc[:, :w], in0=acc[:, :w], in1=st[:, :w]
                    )
                else:  # G
                    _, off, Mstar, members = it
                    T = len(members)
                    st = smallp.tile([P, 8 * D], mybir.dt.bfloat16, tag="sts")
                    blk = rows_h.ap()[off : off + T * Mstar].rearrange(
                        "(t p) f -> p t f", p=Mstar
                    )
                    st3 = st[0:Mstar, : T * D].rearrange("p (t f) -> p t f", f=D)
                    nc.sync.dma_start(out=st3, in_=blk)
                    for t, (r, mr) in enumerate(members):
                        nc.vector.tensor_max(
                            out=acc[0:mr, 0:D],
                            in0=acc[0:mr, 0:D],
                            in1=st[0:mr, t * D : (t + 1) * D],
                        )
                flush(i)
    nc.compile()
    return nc


def kernel(x_locs, x_clusters, edge_src, edge_dst):
    global LAST_RESULTS, LAST_NC
    from concourse.bass_utils import run_bass_kernel_spmd

    x_clusters = np.ascontiguousarray(np.asarray(x_clusters, dtype=np.float32))
    in_maps, order, items, TOT = _host_prep(x_locs, x_clusters, edge_src, edge_dst)
    nc = _build_program(items, TOT)
    LAST_NC = nc
    try:
        res = run_bass_kernel_spmd(nc, in_maps, list(range(N_CORES)))
    except Exception:
        # transient NRT/tunnel faults clear on re-execution; retry once
        res = run_bass_kernel_spmd(nc, in_maps, list(range(N_CORES)))
    LAST_RESULTS = res

    full = np.empty((N_CLUSTERS, 2 * D), dtype=np.float32)
    full[:, :D] = x_clusters
    for k in range(N_CORES):
        o = np.asarray(res.results[k]["out"])  # [P, CHUNKS, D] bf16
        o = o.transpose(1, 0, 2).reshape(CPC, D)  # indexed by sorted rank
        full[order[k], D:] = o.astype(np.float32)
    return full


# revision 29
# speedup vs baseline: 1.0042x; 1.0042x over previous
"""Trainium2 Bass kernel for nn_Loc2Cluster (GNN message passing, segment-max).

Computation: agg[c] = elementwise-max over locs with edge to cluster c of
x_locs[loc]; empty clusters -> 0; output = concat([x_clusters, agg], -1).

Strategy (cluster-sharded, zero collectives, bf16 streams):
  - Core k owns clusters [4096k, 4096(k+1)) after a global count-desc sort
    dealt round-robin across cores (balances per-core round sizes to +-1).
  - Host routes each edge's loc row (pre-converted to bf16; rel tol is
    2e-2, bf16 rounding is ~2^-9) to the core owning its dst cluster.
  - Rows are laid out in "rounds": round r holds the r-th edge row of every
    cluster with count > r, in local-rank order (chunk-major), so each
    round's segment-max is elementwise tensor_max over a prefix of slots.
  - Big rounds (m_r > 128) stream exactly m_r rows: full chunks as one
    chunk-major DMA + the partial last chunk as a small second DMA, each
    followed by its own tensor_max. No pad bytes.
  - Tail rounds (m_r <= 128) all touch only chunk 0; consecutive ones are
    merged into grouped DMA blocks (padded to the group max with -1e30,
    which is max-neutral) and are scheduled right after round 0 so their
    serial max chains hide under the big-round stream DMAs.
  - The last big round is split so the final flush waits only on one small
    [128, 256] tensor_max.
  - Round 0 is DMA'd straight into the accumulator (zero rows for empty
    clusters match the reference's 0-fill).
  - Output chunks are written back (scalar-engine queue) as soon as no
    later item touches them, overlapping writeback with the stream.
  - x_clusters never touches the device: the concat left half is assembled
    on the host at full fp32 precision during unsharding.
"""

import sys

import numpy as np

if "/opt/trn_rl_repo" not in sys.path:
    sys.path.insert(0, "/opt/trn_rl_repo")

import ml_dtypes

BF16 = ml_dtypes.bfloat16

N_LOCS = 262144
N_CLUSTERS = 32768
D = 256
N_CORES = 8
CPC = N_CLUSTERS // N_CORES  # 4096 clusters per core
P = 128
CHUNKS = CPC // P  # 32 chunks of 128 clusters
NEG = np.float32(-1e30)

LAST_RESULTS = None  # BassKernelResults of the most recent run (for profiling)
LAST_NC = None  # compiled Bass module of the most recent run (for TimelineSim)


def _make_schedule(m):
    """Turn per-core round sizes m[r] into an execution schedule.

    Returns (items, offs_of_round, TOT). items are in execution order:
      ("R0",)                      round 0, CPC rows into the accumulator
      ("A", off, mr)               big round, exact mr rows (q full chunks
                                   chunk-major + rem-row partial chunk)
      ("AB", off, q, rem)          partial-chunk piece of the split last round
      ("AA", off, q)               full-chunk piece of the split last round
      ("G", off, Mstar, members)   merged tail rounds, members=[(r, m_r)]
    Stream layout (HBM row offsets) is in round order; execution order is
    [R0, tail groups..., big rounds desc, last round's B then A piece].
    """
    R = len(m)
    offs_of_round = np.zeros(R, dtype=np.int64)
    a_items = []
    g_items = []
    pos = 0
    r = 1
    offs_of_round[0] = 0
    pos = CPC
    while r < R:
        if m[r] > P:
            offs_of_round[r] = pos
            a_items.append(("A", int(pos), int(m[r])))
            pos += int(m[r])
            r += 1
        else:
            # greedy tail group of consecutive small rounds; new group when
            # padding to the group max would get wasteful
            Mstar = int(m[r])
            members = []
            gpos = pos
            while r < R and m[r] <= P and m[r] * 4 >= Mstar and len(members) < 8:
                members.append((r, int(m[r])))
                offs_of_round[r] = gpos
                gpos += Mstar
                r += 1
            g_items.append(("G", int(pos), Mstar, members))
            pos = gpos
    items = [("R0",)] + g_items + a_items
    # split the last big round so the final flush gates on one small max;
    # the partial-chunk piece slots in before the preceding round so its
    # writeback eligibility comes earlier
    if a_items:
        kind, off, mr = a_items[-1]
        q, rem = divmod(mr, P)
        if q > 0:
            items = items[:-1]
            tail = []
            if rem > 0:
                tail.append(("AB", off + q * P, q, rem))
            tail.append(("AA", off, q))
            if rem > 0 and len(a_items) >= 2:
                items = items[:-1] + [tail[0], items[-1]] + tail[1:]
            else:
                items = items + tail
    return items, offs_of_round, int(pos)


def _host_prep(x_locs, x_clusters, edge_src, edge_dst):
    """Build per-core bf16 round-major row streams."""
    x_locs = np.asarray(x_locs, dtype=np.float32)
    x_bf = x_locs.astype(BF16)
    src = np.asarray(edge_src).astype(np.int64)
    dst = np.asarray(edge_dst).astype(np.int64)
    n_edges = dst.shape[0]

    counts = np.bincount(dst, minlength=N_CLUSTERS)  # [32768]

    gorder = np.argsort(-counts, kind="stable")
    grank = np.empty_like(gorder)
    grank[gorder] = np.arange(N_CLUSTERS)
    order = np.ascontiguousarray(gorder.reshape(CPC, N_CORES).T)  # [8, CPC]

    # occurrence index of each edge within its dst cluster
    by_dst = np.argsort(dst, kind="stable")
    group_start = np.zeros(N_CLUSTERS, dtype=np.int64)
    np.cumsum(counts[:-1], out=group_start[1:])
    occ = np.empty(n_edges, dtype=np.int64)
    occ[by_dst] = np.arange(n_edges, dtype=np.int64) - group_start[dst[by_dst]]

    g_of = grank[dst]
    core_of = g_of % N_CORES
    rank_of = g_of // N_CORES  # local rank s, count-sorted desc

    R = max(int(counts.max()), 1)
    counts_sorted = counts[gorder]
    m_r_g = (counts_sorted[None, :] > np.arange(R)[:, None]).sum(axis=1)
    m = (m_r_g + N_CORES - 1) // N_CORES  # shared per-core round size
    m[0] = CPC  # round 0 covers every slot (zeros for empty clusters)

    items, offs_of_round, TOT = _make_schedule(m)

    # chunk-major blocks everywhere: slot within a round block == local rank
    slot = offs_of_round[occ] + rank_of

    slot_src = np.full((N_CORES, TOT), -1, dtype=np.int64)
    slot_src[core_of, slot] = src

    in_maps = []
    for k in range(N_CORES):
        ss = slot_src[k]
        stream = x_bf[np.maximum(ss, 0)]  # [TOT, 256] bf16
        pad = ss < 0
        p0 = np.flatnonzero(pad[:CPC])
        if p0.size:
            stream[p0] = 0.0  # empty clusters -> 0
        pr = np.flatnonzero(pad[CPC:]) + CPC
        if pr.size:
            stream[pr] = NEG  # pads (group + straggler) are max-neutral
        in_maps.append({"rows": np.ascontiguousarray(stream)})

    return in_maps, order, items, TOT


def _build_program(items, TOT, bufs=9):
    from concourse import bacc, mybir
    from concourse._compat import axon_active
    from concourse.tile import TileContext

    nc = bacc.Bacc(
        "TRN2",
        target_bir_lowering=False,
        debug=not axon_active(),
        num_devices=N_CORES,
    )
    rows_h = nc.dram_tensor("rows", [TOT, D], mybir.dt.bfloat16, kind="ExternalInput")
    out_h = nc.dram_tensor(
        "out", [P, CHUNKS, D], mybir.dt.bfloat16, kind="ExternalOutput"
    )

    # per-chunk last-toucher item index, for early writeback
    def touched(it):
        if it[0] == "R0":
            return range(CHUNKS)
        if it[0] == "A":
            q, rem = divmod(it[2], P)
            return range(q + (1 if rem else 0))
        if it[0] == "AB":
            return range(it[2], it[2] + 1)
        if it[0] == "AA":
            return range(it[2])
        return range(1)  # G: chunk 0 only

    last_touch = [0] * CHUNKS
    for i, it in enumerate(items):
        for c in touched(it):
            last_touch[c] = i

    with TileContext(nc) as tc:
        with (
            tc.tile_pool(name="accp", bufs=1) as accp,
            tc.tile_pool(name="stagep", bufs=bufs) as stagep,
            tc.tile_pool(name="smallp", bufs=4) as smallp,
        ):
            acc = accp.tile([P, CHUNKS * D], mybir.dt.bfloat16)
            acc3 = acc[:].rearrange("p (x f) -> p x f", f=D)
            pending_hi = CHUNKS

            def flush(i):
                nonlocal pending_hi
                lo = pending_hi
                while lo > 0 and last_touch[lo - 1] <= i:
                    lo -= 1
                # merge tiny writebacks mid-stream, but flush eagerly near
                # the end so the final flush gates on as little as possible
                eager = i >= len(items) - 4
                final = i == len(items) - 1
                if lo < pending_hi and (eager or pending_hi - lo >= 3):
                    # a DMA's sem wait holds its SEQ: mid-stream writebacks
                    # go on the otherwise-idle scalar queue so they never
                    # stall stream-DMA issue; the final flush goes on SP
                    # (free by then, and its issue path is ~140ns shorter)
                    eng = nc.sync if final else nc.scalar
                    eng.dma_start(
                        out=out_h.ap()[:, lo:pending_hi, :],
                        in_=acc3[:, lo:pending_hi, :],
                    )
                    pending_hi = lo

            def dma_a_piece(dst, off, q):
                blk = rows_h.ap()[off : off + q * P].rearrange(
                    "(x p) f -> p x f", p=P
                )
                dst3 = dst[:, : q * D].rearrange("p (x f) -> p x f", f=D)
                nc.sync.dma_start(out=dst3, in_=blk)

            def dma_b_piece(dst, off, q, rem):
                nc.sync.dma_start(
                    out=dst[0:rem, q * D : (q + 1) * D],
                    in_=rows_h.ap()[off : off + rem],
                )

            for i, it in enumerate(items):
                if it[0] == "R0":
                    dma_a_piece(acc, 0, CHUNKS)
                elif it[0] == "A":
                    _, off, mr = it
                    q, rem = divmod(mr, P)
                    st = stagep.tile([P, CHUNKS * D], mybir.dt.bfloat16, tag="st")
                    if q > 0:
                        dma_a_piece(st, off, q)
                    if rem > 0:
                        dma_b_piece(st, off + q * P, q, rem)
                    if q > 0:
                        w = q * D
                        nc.vector.tensor_max(
                            out=acc[:, :w], in0=acc[:, :w], in1=st[:, :w]
                        )
                    if rem > 0:
                        sl = slice(q * D, (q + 1) * D)
                        nc.vector.tensor_max(
                            out=acc[0:rem, sl], in0=acc[0:rem, sl], in1=st[0:rem, sl]
                        )
                elif it[0] == "AB":
                    _, off, q, rem = it
                    st = smallp.tile([P, 8 * D], mybir.dt.bfloat16, tag="sts")
                    # stage in column 0 of the narrow tile regardless of q
                    nc.sync.dma_start(
                        out=st[0:rem, 0:D], in_=rows_h.ap()[off : off + rem]
                    )
                    sl = slice(q * D, (q + 1) * D)
                    nc.vector.tensor_max(
                        out=acc[0:rem, sl], in0=acc[0:rem, sl], in1=st[0:rem, 0:D]
                    )
                elif it[0] == "AA":
                    _, off, q = it
                    if q <= 8:
                        st = smallp.tile([P, 8 * D], mybir.dt.bfloat16, tag="sts")
                    else:
                        st = stagep.tile([P, CHUNKS * D], mybir.dt.bfloat16, tag="st")
                    dma_a_piece(st, off, q)
                    w = q * D
                    nc.vector.tensor_max(
                        out=acc[:, :w], in0=acc[:, :w], in1=st[:, :w]
                    )
                else:  # G
                    _, off, Mstar, members = it
                    T = len(members)
                    st = smallp.tile([P, 8 * D], mybir.dt.bfloat16, tag="sts")
                    blk = rows_h.ap()[off : off + T * Mstar].rearrange(
                        "(t p) f -> p t f", p=Mstar
                    )
                    st3 = st[0:Mstar, : T * D].rearrange("p (t f) -> p t f", f=D)
                    nc.sync.dma_start(out=st3, in_=blk)
                    for t, (r, mr) in enumerate(members):
                        nc.vector.tensor_max(
                            out=acc[0:mr, 0:D],
                            in0=acc[0:mr, 0:D],
                            in1=st[0:mr, t * D : (t + 1) * D],
                        )
                flush(i)
    nc.compile()
    return nc


def kernel(x_locs, x_clusters, edge_src, edge_dst):
    global LAST_RESULTS, LAST_NC
    from concourse.bass_utils import run_bass_kernel_spmd

    x_clusters = np.ascontiguousarray(np.asarray(x_clusters, dtype=np.float32))
    in_maps, order, items, TOT = _host_prep(x_locs, x_clusters, edge_src, edge_dst)
    nc = _build_program(items, TOT)
    LAST_NC = nc
    try:
        res = run_bass_kernel_spmd(nc, in_maps, list(range(N_CORES)))
    except Exception:
        # transient NRT/tunnel faults clear on re-execution; retry once
        res = run_bass_kernel_spmd(nc, in_maps, list(range(N_CORES)))
    LAST_RESULTS = res

    full = np.empty((N_CLUSTERS, 2 * D), dtype=np.float32)
    full[:, :D] = x_clusters
    for k in range(N_CORES):
        o = np.asarray(res.results[k]["out"])  # [P, CHUNKS, D] bf16
        o = o.transpose(1, 0, 2).reshape(CPC, D)  # indexed by sorted rank
        full[order[k], D:] = o.astype(np.float32)
    return full
